# revision 1
# baseline (speedup 1.0000x reference)
"""MultiHeadAttention (1x1-conv projections) Trainium2 Bass kernel.

Problem: x[8,256,32,32]; q/k/v = conv1x1(x, W*, b*); 8 heads, dk=dv=32;
attention over N=H*W=1024 positions; out = conv1x1(o, Wo, bo).

Sharding: data-parallel over batch -- core c computes batch c.

Per-core dataflow (everything stays on-chip after the initial loads):
  X [256,1024] (C on partitions, 2 tiles of 128)
  q = Wq@X+bq, k = Wk@X+bk       -> [co_part, n]   (co = 32*head+d)
  vT = (Wv@X)^T via x-stationary -> [n_part, co] with a ones column per
       head, so the PV matmul also produces the softmax denominator free
  per head pair: S^T[nk,nq] = k_h^T q_h (K=dk=32; the two heads run
       concurrently in distinct PE row strips via tile_position);
       P^T = exp(scale*S^T) on ScalarE -- the bottleneck engine: 67M exps
       across the batch = 8.4M/core ~ 55us at 128 lanes * 1.2 GHz;
       o_h[dv+1, nq] = [vT_h|1]^T P^T accumulated over nk tiles in PSUM
  o_norm = o / denom ; y = Wo@o_norm + (Wo@bv + bo)

Softmax max-subtraction is skipped: logits ~ N(0,1) so fp32 exp() cannot
overflow, and softmax is shift-invariant (identical value). bv is folded
into the output bias (Wo@bv + bo), computed on-device. The reciprocal of
the denominator row is partition-broadcast with a K=1 ones-matmul.

Matmuls run as float32r (full-rate fp32 on the PE at free-dim >= 256;
measured rel err vs fp32 reference ~4e-4).

PSUM budget (8 banks): qk 2x[128,1024]=4 (double-buffered so QK overlaps
exp), pv 3x[33,512]=3, shared [128,512] slot (projections / reciprocal
broadcast / output projection) = 1.

All engines execute their streams strictly in-order, so the emission is
software-pipelined by hand: each pass's last two PV pairs and its
normalize epilogue are emitted inside the NEXT pass (after its first
QK/exp), q/k/v projections are interleaved into the seams, and each
nq-half's output projection + store overlap the other half's attention.
ScalarE then runs its 64 exp instructions back-to-back (~86% of the
kernel), which is the compute roofline for this op.
"""

import numpy as np

import concourse.bass as bass
import concourse.bacc as bacc
import concourse.mybir as mybir
import concourse.tile as tile
from concourse.bass_utils import run_bass_kernel_spmd

F32 = mybir.dt.float32
F32R = mybir.dt.float32r
AF = mybir.ActivationFunctionType

P = 128
C = 256          # channels (= Ck = Cv = Co)
CT = 2           # channel tiles of 128
N = 1024         # sequence length (H*W)
NH = 8           # heads
DK = 32          # head dim
SCALE = DK ** -0.5
NQH = 2          # nq halves (512 each; fp32 matmul free-dim limit)
NKT = 8          # nk tiles of 128


def build_nc(reps=1, pipelined=True):
    nc = bacc.Bacc(None, target_bir_lowering=False, debug=False)

    x_d = nc.dram_tensor("x", [C, N], F32R, kind="ExternalInput")
    wqt_d = nc.dram_tensor("wqt", [C, C], F32R, kind="ExternalInput")
    wkt_d = nc.dram_tensor("wkt", [C, C], F32R, kind="ExternalInput")
    wvt_d = nc.dram_tensor("wvt", [C, C], F32R, kind="ExternalInput")
    wot_d = nc.dram_tensor("wot", [C, C], F32R, kind="ExternalInput")
    bq_d = nc.dram_tensor("bq", [C], F32, kind="ExternalInput")
    bk_d = nc.dram_tensor("bk", [C], F32, kind="ExternalInput")
    bv_d = nc.dram_tensor("bv", [C], F32R, kind="ExternalInput")
    bo_d = nc.dram_tensor("bo", [C], F32, kind="ExternalInput")
    y_d = nc.dram_tensor("y", [C, N], F32, kind="ExternalOutput")

    with tile.TileContext(nc) as tc:
        with (
            tc.tile_pool(name="const", bufs=1) as cpool,
            tc.tile_pool(name="work", bufs=1) as wpool,
            tc.tile_pool(name="qkpsum", bufs=2, space="PSUM") as qkpool,
            tc.tile_pool(name="pvpsum", bufs=3, space="PSUM") as pvpool,
            tc.tile_pool(name="mmpsum", bufs=1, space="PSUM") as mmpool,
            tc.tile_pool(name="ptpool", bufs=6) as ptpool,
            tc.tile_pool(name="eppool", bufs=3) as eppool,
        ):
            # ---- loads ----
            # spread across three DMA queues so the first q/k projections
            # (and with them the first exp) start as early as possible:
            #   sync:   wq, x[ci0], x[ci1]
            #   gpsimd: wk, bq, bk, wv, wo, bo, bv2
            w_s = {}
            for name, d in (("q", wqt_d), ("k", wkt_d), ("v", wvt_d), ("o", wot_d)):
                w_s[name] = cpool.tile([P, CT, C], F32R, tag=f"w{name}",
                                       name=f"w{name}")
            x_s = cpool.tile([P, CT, N], F32R)
            xr = x_d[:].rearrange("(t p) n -> p t n", p=P)
            # x quarters split across two queues; the first q-projection only
            # needs the n<512 halves of both ci tiles
            for nh in range(NQH):
                nc.sync.dma_start(
                    x_s[:, 0, nh * 512 : (nh + 1) * 512],
                    xr[:, 0, nh * 512 : (nh + 1) * 512],
                )
                nc.scalar.dma_start(
                    x_s[:, 1, nh * 512 : (nh + 1) * 512],
                    xr[:, 1, nh * 512 : (nh + 1) * 512],
                )
            nc.gpsimd.dma_start(
                w_s["q"][:], wqt_d[:].rearrange("(t p) c -> p t c", p=P)
            )
            nc.gpsimd.dma_start(
                w_s["k"][:], wkt_d[:].rearrange("(t p) c -> p t c", p=P)
            )
            b_s = {}
            for name, d in (("q", bq_d), ("k", bk_d), ("o", bo_d)):
                b_s[name] = cpool.tile([P, CT], F32, tag=f"b{name}",
                                       name=f"b{name}")
                nc.gpsimd.dma_start(b_s[name][:], d[:].rearrange("(t p) -> p t", p=P))
            nc.gpsimd.dma_start(
                w_s["v"][:], wvt_d[:].rearrange("(t p) c -> p t c", p=P)
            )
            nc.gpsimd.dma_start(
                w_s["o"][:], wot_d[:].rearrange("(t p) c -> p t c", p=P)
            )
            # bv duplicated along a free dim of 2: fp32r matmuls need N>=2
            bv2_s = cpool.tile([P, CT, 2], F32R, tag="bv2")
            for j in range(2):
                nc.gpsimd.dma_start(
                    bv2_s[:, :, j], bv_d[:].rearrange("(t p) -> p t", p=P)
                )

            # ---- persistent working tiles ----
            q_s = wpool.tile([P, CT, N], F32R, tag="q")     # [co_p, co_t, n]
            k_s = wpool.tile([P, CT, N], F32R, tag="k")
            # vT with a ones column per head: [n_p, n_t, head, dv+1]
            vt_s = wpool.tile([P, NKT, NH, DK + 1], F32R, tag="vt")
            o_s = wpool.tile([P, CT, N], F32R, tag="o")     # normalized attn out
            y_s = wpool.tile([P, CT, N], F32, tag="y")
            bo2_s = wpool.tile([P, CT], F32, tag="bo2")     # Wo@bv + bo

            nc.vector.memset(vt_s[:, :, :, DK : DK + 1].bitcast(F32), 1.0)
            ones1 = wpool.tile([1, DK], F32R, tag="ones1")
            nc.vector.memset(ones1[:].bitcast(F32), 1.0)
            # pull the exp ACT-table load into the load phase
            warm = wpool.tile([1, 2], F32, tag="warm")
            nc.scalar.activation(warm[:], ones1[:, 0:2], AF.Exp)

            # ---- emission helpers ----
            def qk_proj(name, dst, ct, nh):
                ps = mmpool.tile([P, 512], F32, tag="mm512", name="ps")
                for ci in range(CT):
                    nc.tensor.matmul(
                        ps[:],
                        w_s[name][:, ci, ct * P : (ct + 1) * P],
                        x_s[:, ci, nh * 512 : (nh + 1) * 512],
                        start=(ci == 0),
                        stop=(ci == CT - 1),
                    )
                nc.vector.tensor_scalar_add(
                    dst[:, ct, nh * 512 : (nh + 1) * 512],
                    ps[:],
                    b_s[name][:, ct : ct + 1],
                )

            def v_proj(nt):
                ps = mmpool.tile([P, 512], F32, tag="mm512", name="ps")
                for ci in range(CT):
                    nc.tensor.matmul(
                        ps[:, 0:C],
                        x_s[:, ci, nt * P : (nt + 1) * P],
                        w_s["v"][:, ci, :],
                        start=(ci == 0),
                        stop=(ci == CT - 1),
                    )
                nc.vector.tensor_copy(
                    vt_s[:, nt, :, 0:DK],
                    ps[:, 0:C].rearrange("p (h d) -> p h d", d=DK),
                )

            def bo2_proj():
                # bo2 = WoT.T @ bv + bo
                for ct in range(CT):
                    ps = mmpool.tile([P, 512], F32, tag="mm512", name="ps")
                    for ci in range(CT):
                        nc.tensor.matmul(
                            ps[:, 0:2],
                            w_s["o"][:, ci, ct * P : (ct + 1) * P],
                            bv2_s[:, ci, :],
                            start=(ci == 0),
                            stop=(ci == CT - 1),
                        )
                    nc.vector.tensor_scalar_add(
                        bo2_s[:, ct : ct + 1], ps[:, 0:1],
                        b_s["o"][:, ct : ct + 1]
                    )

            def attn_pass(nqh, pp, pre_pv=None, tail_prev=None, epi_prev=None,
                          last=False):
                """One pass = 2 heads (4*hg + hl0, +1) x one nq-half.

                The PE stream is strictly in-order, so anything gated on this
                pass's LAST exps must come after the next pass's first QKs in
                the stream or ScalarE idles at the boundary.  The last two
                nk's PV matmuls are returned as `pv_tail` (emitted at nk==0
                of the next pass, right after its first QK), and the
                normalize epilogue as `epilogue` (emitted at nk==3, when the
                DVE reciprocal chain is ready and PE has QKs in flight).
                """
                hg = pp // 2
                hl0 = (pp % 2) * 2
                pvs = [
                    pvpool.tile([DK + 1, 512], F32, tag="pv", name=f"pv{j}")
                    for j in range(2)
                ]
                pt_hold = {}
                for nk in range(NKT):
                    qk = qkpool.tile([P, 1024], F32, tag="qk")
                    for j in range(2):
                        hl = hl0 + j
                        nc.tensor.matmul(
                            qk[:, j * 512 : (j + 1) * 512],
                            k_s[hl * DK : (hl + 1) * DK, hg,
                                nk * P : (nk + 1) * P],
                            q_s[hl * DK : (hl + 1) * DK, hg,
                                nqh * 512 : (nqh + 1) * 512],
                            start=True,
                            stop=True,
                            tile_position=(hl * DK, 0),
                        )
                    pt = ptpool.tile([P, 1024], F32R, tag="pt")
                    nc.scalar.activation(pt[:], qk[:], AF.Exp, scale=SCALE)
                    if nk == 0 and tail_prev is not None:
                        tail_prev()
                    if nk == 3 and epi_prev is not None:
                        epi_prev()
                    if pre_pv is not None:
                        pre_pv(nk)
                    if nk < NKT - 2:
                        for j in range(2):
                            nc.tensor.matmul(
                                pvs[j][:],
                                vt_s[:, nk, 4 * hg + hl0 + j, :],
                                pt[:, j * 512 : (j + 1) * 512],
                                start=(nk == 0),
                                stop=False,
                            )
                    else:
                        pt_hold[nk] = pt

                def pv_tail():
                    # complete head j=0's accumulator first so the epilogue
                    # chain for it starts one matmul earlier
                    for j in range(2):
                        for nk in (NKT - 2, NKT - 1):
                            nc.tensor.matmul(
                                pvs[j][:],
                                vt_s[:, nk, 4 * hg + hl0 + j, :],
                                pt_hold[nk][:, j * 512 : (j + 1) * 512],
                                start=False,
                                stop=(nk == NKT - 1),
                            )

                def epilogue():
                    # evacuate PSUM, normalize o_h by 1/denom. The reciprocal
                    # row is partition-broadcast with a K=1 ones-matmul on
                    # the PE into a shared PSUM slot.
                    for j in range(2):
                        hl = hl0 + j
                        oraw = eppool.tile([DK + 1, 512], F32, tag="oraw")
                        if last:
                            nc.scalar.copy(oraw[:], pvs[j][:])
                        else:
                            nc.vector.tensor_copy(oraw[:], pvs[j][:])
                        rec = eppool.tile([1, 512], F32R, tag="rec")
                        with nc.allow_low_precision(reason="f32r bcast"):
                            nc.vector.reciprocal(rec[:], oraw[DK : DK + 1, :])
                        bcp = mmpool.tile([P, 512], F32, tag="mm512",
                                          name="bcp")
                        nc.tensor.matmul(
                            bcp[0:DK, :], ones1[:], rec[:], start=True,
                            stop=True
                        )
                        nc.vector.tensor_tensor(
                            o_s[hl * DK : (hl + 1) * DK, hg,
                                nqh * 512 : (nqh + 1) * 512],
                            oraw[0:DK, :],
                            bcp[0:DK, :],
                            mybir.AluOpType.mult,
                        )

                return pv_tail, epilogue

            def out_proj(nqh, last=False):
                # output projection + store for one nq-half
                for ct in range(CT):
                    ps = mmpool.tile([P, 512], F32, tag="mm512", name="ps")
                    for i, cv in enumerate((1, 0)):
                        nc.tensor.matmul(
                            ps[:],
                            w_s["o"][:, cv, ct * P : (ct + 1) * P],
                            o_s[:, cv, nqh * 512 : (nqh + 1) * 512],
                            start=(i == 0),
                            stop=(i == CT - 1),
                        )
                    if last:
                        nc.scalar.activation(
                            y_s[:, ct, nqh * 512 : (nqh + 1) * 512],
                            ps[:],
                            AF.Identity,
                            bias=bo2_s[:, ct : ct + 1],
                        )
                    else:
                        nc.vector.tensor_scalar_add(
                            y_s[:, ct, nqh * 512 : (nqh + 1) * 512],
                            ps[:],
                            bo2_s[:, ct : ct + 1],
                        )
                    nc.sync.dma_start(
                        y_d[:].rearrange("(t p) n -> p t n", p=P)[
                            :, ct, nqh * 512 : (nqh + 1) * 512
                        ],
                        y_s[:, ct, nqh * 512 : (nqh + 1) * 512],
                    )

            # ---- emission order: overlap projections with attention ----
            # Within each nq-half run ct1 head-pairs (pp 2,3) before ct0
            # (pp 0,1) so the output projection's cv=ct1 operand is ready
            # early and the projection finishes right after the last pass.
            for _rep in range(reps):
                if pipelined:
                    qk_proj("q", q_s, 1, 0)
                    qk_proj("k", k_s, 1, 0)
                    qk_proj("k", k_s, 1, 1)
                    tailp, epip = attn_pass(0, 2, pre_pv=v_proj)
                    qk_proj("q", q_s, 0, 0)
                    qk_proj("k", k_s, 0, 0)
                    tailp, epip = attn_pass(0, 3, tail_prev=tailp, epi_prev=epip)
                    qk_proj("k", k_s, 0, 1)
                    qk_proj("q", q_s, 1, 1)
                    tailp, epip = attn_pass(0, 0, tail_prev=tailp, epi_prev=epip)
                    qk_proj("q", q_s, 0, 1)
                    bo2_proj()
                    tailp, epip = attn_pass(0, 1, tail_prev=tailp, epi_prev=epip)

                    def epi_and_oproj0(epip=epip):
                        epip()
                        out_proj(0)

                    tailp, epip = attn_pass(1, 2, tail_prev=tailp,
                                            epi_prev=epi_and_oproj0)
                    tailp, epip = attn_pass(1, 3, tail_prev=tailp, epi_prev=epip)
                    tailp, epip = attn_pass(1, 0, tail_prev=tailp, epi_prev=epip)
                    tailp, epip = attn_pass(1, 1, tail_prev=tailp,
                                            epi_prev=epip, last=True)
                    tailp()
                    epip()
                    out_proj(1, last=True)
                else:
                    qk_proj("q", q_s, 1, 0)
                    qk_proj("k", k_s, 1, 0)
                    qk_proj("k", k_s, 1, 1)
                    tailp, epip = attn_pass(0, 2, pre_pv=v_proj)
                    tailp(); epip()
                    qk_proj("q", q_s, 0, 0)
                    qk_proj("k", k_s, 0, 0)
                    qk_proj("k", k_s, 0, 1)
                    tailp, epip = attn_pass(0, 3)
                    tailp(); epip()
                    qk_proj("q", q_s, 1, 1)
                    tailp, epip = attn_pass(0, 0)
                    tailp(); epip()
                    qk_proj("q", q_s, 0, 1)
                    bo2_proj()
                    tailp, epip = attn_pass(0, 1)
                    tailp(); epip()
                    out_proj(0)
                    tailp, epip = attn_pass(1, 2)
                    tailp(); epip()
                    tailp, epip = attn_pass(1, 3)
                    tailp(); epip()
                    tailp, epip = attn_pass(1, 0)
                    tailp(); epip()
                    tailp, epip = attn_pass(1, 1, last=True)
                    tailp(); epip()
                    out_proj(1, last=True)
    nc.compile()
    return nc


_NC = None


def _get_nc():
    global _NC
    if _NC is None:
        _NC = build_nc()
    return _NC


def make_in_maps(x, Wq, bq, Wk, bk, Wv, bv, Wo, bo):
    B = x.shape[0]
    xs = np.ascontiguousarray(x.reshape(B, C, N).astype(np.float32, copy=False))
    shared = {
        "wqt": np.ascontiguousarray(Wq.T.astype(np.float32, copy=False)),
        "wkt": np.ascontiguousarray(Wk.T.astype(np.float32, copy=False)),
        "wvt": np.ascontiguousarray(Wv.T.astype(np.float32, copy=False)),
        "wot": np.ascontiguousarray(Wo.T.astype(np.float32, copy=False)),
        "bq": np.ascontiguousarray(bq.astype(np.float32, copy=False)),
        "bk": np.ascontiguousarray(bk.astype(np.float32, copy=False)),
        "bv": np.ascontiguousarray(bv.astype(np.float32, copy=False)),
        "bo": np.ascontiguousarray(bo.astype(np.float32, copy=False)),
    }
    return [dict(shared, x=xs[c]) for c in range(B)]


def kernel(x, Wq, bq, Wk, bk, Wv, bv, Wo, bo, **run_kwargs):
    x = np.asarray(x)
    B, _, H, W = x.shape
    in_maps = make_in_maps(
        x, np.asarray(Wq), np.asarray(bq), np.asarray(Wk), np.asarray(bk),
        np.asarray(Wv), np.asarray(bv), np.asarray(Wo), np.asarray(bo),
    )
    res = run_bass_kernel_spmd(_get_nc(), in_maps, core_ids=list(range(B)),
                               **run_kwargs)
    y = np.stack([res.results[c]["y"] for c in range(B)])
    out = y.reshape(B, C, H, W)
    if run_kwargs:
        kernel.last_result = res
    return out



# revision 11
# speedup vs baseline: 698.6530x; 698.6530x over previous
"""MultiHeadAttention (1x1-conv projections) Trainium2 Bass kernel.

Problem: x[8,256,32,32]; q/k/v = conv1x1(x, W*, b*); 8 heads, dk=dv=32;
attention over N=H*W=1024 positions; out = conv1x1(o, Wo, bo).

Sharding: data-parallel over batch -- core c computes batch c.

Per-core dataflow (everything stays on-chip after the initial loads):
  X [256,1024] (C on partitions, 2 tiles of 128)
  q = Wq@X+bq, k = Wk@X+bk       -> [co_part, n]   (co = 32*head+d)
  vT = (Wv@X)^T via x-stationary -> [n_part, co] with a ones column per
       head, so the PV matmul also produces the softmax denominator free
  per head pair: S^T[nk,nq] = k_h^T q_h (K=dk=32; the two heads run
       concurrently in distinct PE row strips via tile_position);
       P^T = exp(scale*S^T) on ScalarE -- the bottleneck engine: 67M exps
       across the batch = 8.4M/core ~ 55us at 128 lanes * 1.2 GHz;
       o_h[dv+1, nq] = [vT_h|1]^T P^T accumulated over nk tiles in PSUM
  o_norm = o / denom ; y = Wo@o_norm + (Wo@bv + bo)

Softmax max-subtraction is skipped: logits ~ N(0,1) so fp32 exp() cannot
overflow, and softmax is shift-invariant (identical value). bv is folded
into the output bias (Wo@bv + bo), computed on-device. The reciprocal of
the denominator row is partition-broadcast with a K=1 ones-matmul.

Matmuls run as float32r (full-rate fp32 on the PE at free-dim >= 256;
measured rel err vs fp32 reference ~4e-4).

PSUM budget (8 banks): qk 2x[128,1024]=4 (double-buffered so QK overlaps
exp), pv 3x[33,512]=3, shared [128,512] slot (projections / reciprocal
broadcast / output projection) = 1.

All engines execute their streams strictly in-order, so the emission is
software-pipelined by hand: each pass's last two PV pairs and its
normalize epilogue are emitted inside the NEXT pass (after its first
QK/exp), q/k/v projections are interleaved into the seams, and each
nq-half's output projection + store overlap the other half's attention.
ScalarE then runs its 64 exp instructions back-to-back (~86% of the
kernel), which is the compute roofline for this op.
"""

import contextlib

import numpy as np

import concourse.bass as bass
import concourse.bacc as bacc
import concourse.mybir as mybir
import concourse.tile as tile
from concourse.bass_utils import run_bass_kernel_spmd

F32 = mybir.dt.float32
F32R = mybir.dt.float32r
BF16 = mybir.dt.bfloat16
AF = mybir.ActivationFunctionType

P = 128
C = 256          # channels (= Ck = Cv = Co)
CT = 2           # channel tiles of 128
N = 1024         # sequence length (H*W)
NH = 8           # heads
DK = 32          # head dim
SCALE = DK ** -0.5
NQH = 2          # nq halves (512 each; fp32 matmul free-dim limit)
NKT = 8          # nk tiles of 128


def build_nc(reps=1, pipelined=True, hwloop=1):
    """hwloop>1 wraps the body in a hardware For_i loop (constant NEFF
    size) — used only for device timing, never by kernel()."""
    nc = bacc.Bacc(None, target_bir_lowering=False, debug=False)

    x_d = nc.dram_tensor("x", [C, N], F32R, kind="ExternalInput")
    wqt_d = nc.dram_tensor("wqt", [C, C], F32R, kind="ExternalInput")
    wkt_d = nc.dram_tensor("wkt", [C, C], F32R, kind="ExternalInput")
    wvt_d = nc.dram_tensor("wvt", [C, C], F32R, kind="ExternalInput")
    wot_d = nc.dram_tensor("wot", [C, C], F32R, kind="ExternalInput")
    bq_d = nc.dram_tensor("bq", [C], F32, kind="ExternalInput")
    bk_d = nc.dram_tensor("bk", [C], F32, kind="ExternalInput")
    bv_d = nc.dram_tensor("bv", [C], F32R, kind="ExternalInput")
    bo_d = nc.dram_tensor("bo", [C], F32, kind="ExternalInput")
    y_d = nc.dram_tensor("y", [C, N], F32, kind="ExternalOutput")

    with tile.TileContext(nc) as tc:
        with (
            tc.tile_pool(name="const", bufs=1) as cpool,
            tc.tile_pool(name="work", bufs=1) as wpool,
            tc.tile_pool(name="qkpsum", bufs=2, space="PSUM") as qkpool,
            tc.tile_pool(name="pvpsum", bufs=3, space="PSUM") as pvpool,
            tc.tile_pool(name="mmpsum", bufs=1, space="PSUM") as mmpool,
            tc.tile_pool(name="ptpool", bufs=6) as ptpool,
            tc.tile_pool(name="eppool", bufs=3) as eppool,
        ):
            # ---- loads ----
            # spread across three DMA queues so the first q/k projections
            # (and with them the first exp) start as early as possible:
            #   sync:   wq, x[ci0], x[ci1]
            #   gpsimd: wk, bq, bk, wv, wo, bo, bv2
            w_s = {}
            for name, d in (("q", wqt_d), ("k", wkt_d), ("v", wvt_d), ("o", wot_d)):
                w_s[name] = cpool.tile([P, CT, C], F32R, tag=f"w{name}",
                                       name=f"w{name}")
            x_s = cpool.tile([P, CT, N], F32R)
            xr = x_d[:].rearrange("(t p) n -> p t n", p=P)
            # x quarters split across two queues; the first q-projection only
            # needs the n<512 halves of both ci tiles
            for nh in range(NQH):
                nc.sync.dma_start(
                    x_s[:, 0, nh * 512 : (nh + 1) * 512],
                    xr[:, 0, nh * 512 : (nh + 1) * 512],
                )
                nc.scalar.dma_start(
                    x_s[:, 1, nh * 512 : (nh + 1) * 512],
                    xr[:, 1, nh * 512 : (nh + 1) * 512],
                )
            nc.gpsimd.dma_start(
                w_s["q"][:], wqt_d[:].rearrange("(t p) c -> p t c", p=P)
            )
            nc.gpsimd.dma_start(
                w_s["k"][:], wkt_d[:].rearrange("(t p) c -> p t c", p=P)
            )
            b_s = {}
            for name, d in (("q", bq_d), ("k", bk_d), ("o", bo_d)):
                b_s[name] = cpool.tile([P, CT], F32, tag=f"b{name}",
                                       name=f"b{name}")
                nc.gpsimd.dma_start(b_s[name][:], d[:].rearrange("(t p) -> p t", p=P))
            nc.gpsimd.dma_start(
                w_s["v"][:], wvt_d[:].rearrange("(t p) c -> p t c", p=P)
            )
            nc.gpsimd.dma_start(
                w_s["o"][:], wot_d[:].rearrange("(t p) c -> p t c", p=P)
            )
            # bv duplicated along a free dim of 2: fp32r matmuls need N>=2
            bv2_s = cpool.tile([P, CT, 2], F32R, tag="bv2")
            for j in range(2):
                nc.gpsimd.dma_start(
                    bv2_s[:, :, j], bv_d[:].rearrange("(t p) -> p t", p=P)
                )

            # ---- persistent working tiles ----
            q_s = wpool.tile([P, CT, N], F32R, tag="q")     # [co_p, co_t, n]
            k_s = wpool.tile([P, CT, N], F32R, tag="k")
            # vT with a ones column per head: [n_p, n_t, head, dv+1]
            # bf16: the PV matmul operand dtype; P (post-exp, in [0,e^5])
            # and V quantization errors stay ~0.4% after the 1024-term sum
            vt_s = wpool.tile([P, NKT, NH, DK + 1], BF16, tag="vt")
            o_s = wpool.tile([P, CT, N], F32R, tag="o")     # normalized attn out
            y_s = wpool.tile([P, CT, N], F32, tag="y")
            bo2_s = wpool.tile([P, CT], F32, tag="bo2")     # Wo@bv + bo

            nc.vector.memset(vt_s[:, :, :, DK : DK + 1], 1.0)
            ones1 = wpool.tile([1, DK], F32R, tag="ones1")
            nc.vector.memset(ones1[:].bitcast(F32), 1.0)
            # pull the exp ACT-table load into the load phase
            warm = wpool.tile([1, 2], F32, tag="warm")
            nc.scalar.activation(warm[:], ones1[:, 0:2], AF.Exp)

            # ---- emission helpers ----
            def qk_proj(name, dst, ct, nh):
                ps = mmpool.tile([P, 512], F32, tag="mm512", name="ps")
                for ci in range(CT):
                    nc.tensor.matmul(
                        ps[:],
                        w_s[name][:, ci, ct * P : (ct + 1) * P],
                        x_s[:, ci, nh * 512 : (nh + 1) * 512],
                        start=(ci == 0),
                        stop=(ci == CT - 1),
                    )
                nc.vector.tensor_scalar_add(
                    dst[:, ct, nh * 512 : (nh + 1) * 512],
                    ps[:],
                    b_s[name][:, ct : ct + 1],
                )

            def v_proj(nt):
                ps = mmpool.tile([P, 512], F32, tag="mm512", name="ps")
                for ci in range(CT):
                    nc.tensor.matmul(
                        ps[:, 0:C],
                        x_s[:, ci, nt * P : (nt + 1) * P],
                        w_s["v"][:, ci, :],
                        start=(ci == 0),
                        stop=(ci == CT - 1),
                    )
                nc.vector.tensor_copy(
                    vt_s[:, nt, :, 0:DK],
                    ps[:, 0:C].rearrange("p (h d) -> p h d", d=DK),
                )

            def bo2_proj():
                # bo2 = WoT.T @ bv + bo
                for ct in range(CT):
                    ps = mmpool.tile([P, 512], F32, tag="mm512", name="ps")
                    for ci in range(CT):
                        nc.tensor.matmul(
                            ps[:, 0:2],
                            w_s["o"][:, ci, ct * P : (ct + 1) * P],
                            bv2_s[:, ci, :],
                            start=(ci == 0),
                            stop=(ci == CT - 1),
                        )
                    nc.vector.tensor_scalar_add(
                        bo2_s[:, ct : ct + 1], ps[:, 0:1],
                        b_s["o"][:, ct : ct + 1]
                    )

            def attn_pass(nqh, pp, pre_pv=None, tail_prev=None, epi_prev=None,
                          last=False):
                """One pass = 2 heads (4*hg + hl0, +1) x one nq-half.

                The PE stream is strictly in-order, so anything gated on this
                pass's LAST exps must come after the next pass's first QKs in
                the stream or ScalarE idles at the boundary.  The last two
                nk's PV matmuls are returned as `pv_tail` (emitted at nk==0
                of the next pass, right after its first QK), and the
                normalize epilogue as `epilogue` (emitted at nk==3, when the
                DVE reciprocal chain is ready and PE has QKs in flight).
                """
                hg = pp // 2
                hl0 = (pp % 2) * 2
                pvs = [
                    pvpool.tile([DK + 1, 512], F32, tag="pv", name=f"pv{j}")
                    for j in range(2)
                ]
                pt_hold = {}
                for nk in range(NKT):
                    qk = qkpool.tile([P, 1024], F32, tag="qk")
                    for j in range(2):
                        hl = hl0 + j
                        nc.tensor.matmul(
                            qk[:, j * 512 : (j + 1) * 512],
                            k_s[hl * DK : (hl + 1) * DK, hg,
                                nk * P : (nk + 1) * P],
                            q_s[hl * DK : (hl + 1) * DK, hg,
                                nqh * 512 : (nqh + 1) * 512],
                            start=True,
                            stop=True,
                            tile_position=(hl * DK, 0),
                        )
                    pt = ptpool.tile([P, 1024], BF16, tag="pt")
                    # bf16 output: ACT exp measures ~1.15us vs ~1.79us for
                    # fp32 out on HW -- the single largest cost in the kernel
                    nc.scalar.activation(pt[:], qk[:], AF.Exp, scale=SCALE)
                    if nk == 0 and tail_prev is not None:
                        tail_prev()
                    if nk == 3 and epi_prev is not None:
                        epi_prev()
                    if pre_pv is not None:
                        pre_pv(nk)
                    if nk < NKT - 2:
                        for j in range(2):
                            nc.tensor.matmul(
                                pvs[j][:],
                                vt_s[:, nk, 4 * hg + hl0 + j, :],
                                pt[:, j * 512 : (j + 1) * 512],
                                start=(nk == 0),
                                stop=False,
                            )
                    else:
                        pt_hold[nk] = pt

                def pv_tail():
                    # complete head j=0's accumulator first so the epilogue
                    # chain for it starts one matmul earlier
                    for j in range(2):
                        for nk in (NKT - 2, NKT - 1):
                            nc.tensor.matmul(
                                pvs[j][:],
                                vt_s[:, nk, 4 * hg + hl0 + j, :],
                                pt_hold[nk][:, j * 512 : (j + 1) * 512],
                                start=False,
                                stop=(nk == NKT - 1),
                            )

                def epilogue():
                    # evacuate PSUM, normalize o_h by 1/denom. The reciprocal
                    # row is partition-broadcast with a K=1 ones-matmul on
                    # the PE into a shared PSUM slot.
                    for j in range(2):
                        hl = hl0 + j
                        oraw = eppool.tile([DK + 1, 512], F32, tag="oraw")
                        if last:
                            nc.scalar.copy(oraw[:], pvs[j][:])
                        else:
                            nc.vector.tensor_copy(oraw[:], pvs[j][:])
                        rec = eppool.tile([1, 512], F32R, tag="rec")
                        with nc.allow_low_precision(reason="f32r bcast"):
                            nc.vector.reciprocal(rec[:], oraw[DK : DK + 1, :])
                        bcp = mmpool.tile([P, 512], F32, tag="mm512",
                                          name="bcp")
                        nc.tensor.matmul(
                            bcp[0:DK, :], ones1[:], rec[:], start=True,
                            stop=True
                        )
                        nc.vector.tensor_tensor(
                            o_s[hl * DK : (hl + 1) * DK, hg,
                                nqh * 512 : (nqh + 1) * 512],
                            oraw[0:DK, :],
                            bcp[0:DK, :],
                            mybir.AluOpType.mult,
                        )

                return pv_tail, epilogue

            def out_proj(nqh, last=False):
                # output projection + store for one nq-half
                for ct in range(CT):
                    ps = mmpool.tile([P, 512], F32, tag="mm512", name="ps")
                    for i, cv in enumerate((1, 0)):
                        nc.tensor.matmul(
                            ps[:],
                            w_s["o"][:, cv, ct * P : (ct + 1) * P],
                            o_s[:, cv, nqh * 512 : (nqh + 1) * 512],
                            start=(i == 0),
                            stop=(i == CT - 1),
                        )
                    if last:
                        nc.scalar.activation(
                            y_s[:, ct, nqh * 512 : (nqh + 1) * 512],
                            ps[:],
                            AF.Identity,
                            bias=bo2_s[:, ct : ct + 1],
                        )
                    else:
                        nc.vector.tensor_scalar_add(
                            y_s[:, ct, nqh * 512 : (nqh + 1) * 512],
                            ps[:],
                            bo2_s[:, ct : ct + 1],
                        )
                    nc.sync.dma_start(
                        y_d[:].rearrange("(t p) n -> p t n", p=P)[
                            :, ct, nqh * 512 : (nqh + 1) * 512
                        ],
                        y_s[:, ct, nqh * 512 : (nqh + 1) * 512],
                    )

            # ---- emission order: overlap projections with attention ----
            # Within each nq-half run ct1 head-pairs (pp 2,3) before ct0
            # (pp 0,1) so the output projection's cv=ct1 operand is ready
            # early and the projection finishes right after the last pass.
            loop_ctx = (
                tc.For_i(0, hwloop) if hwloop > 1 else contextlib.nullcontext()
            )
            with loop_ctx:
              for _rep in range(reps):
                if pipelined:
                    qk_proj("q", q_s, 1, 0)
                    qk_proj("k", k_s, 1, 0)
                    qk_proj("k", k_s, 1, 1)
                    tailp, epip = attn_pass(0, 2, pre_pv=v_proj)
                    qk_proj("q", q_s, 0, 0)
                    qk_proj("k", k_s, 0, 0)
                    tailp, epip = attn_pass(0, 3, tail_prev=tailp, epi_prev=epip)
                    qk_proj("k", k_s, 0, 1)
                    qk_proj("q", q_s, 1, 1)
                    tailp, epip = attn_pass(0, 0, tail_prev=tailp, epi_prev=epip)
                    qk_proj("q", q_s, 0, 1)
                    bo2_proj()
                    tailp, epip = attn_pass(0, 1, tail_prev=tailp, epi_prev=epip)

                    def epi_and_oproj0(epip=epip):
                        epip()
                        out_proj(0)

                    tailp, epip = attn_pass(1, 2, tail_prev=tailp,
                                            epi_prev=epi_and_oproj0)
                    tailp, epip = attn_pass(1, 3, tail_prev=tailp, epi_prev=epip)
                    tailp, epip = attn_pass(1, 0, tail_prev=tailp, epi_prev=epip)
                    tailp, epip = attn_pass(1, 1, tail_prev=tailp,
                                            epi_prev=epip, last=True)
                    tailp()
                    epip()
                    out_proj(1, last=True)
                else:
                    qk_proj("q", q_s, 1, 0)
                    qk_proj("k", k_s, 1, 0)
                    qk_proj("k", k_s, 1, 1)
                    tailp, epip = attn_pass(0, 2, pre_pv=v_proj)
                    tailp(); epip()
                    qk_proj("q", q_s, 0, 0)
                    qk_proj("k", k_s, 0, 0)
                    qk_proj("k", k_s, 0, 1)
                    tailp, epip = attn_pass(0, 3)
                    tailp(); epip()
                    qk_proj("q", q_s, 1, 1)
                    tailp, epip = attn_pass(0, 0)
                    tailp(); epip()
                    qk_proj("q", q_s, 0, 1)
                    bo2_proj()
                    tailp, epip = attn_pass(0, 1)
                    tailp(); epip()
                    out_proj(0)
                    tailp, epip = attn_pass(1, 2)
                    tailp(); epip()
                    tailp, epip = attn_pass(1, 3)
                    tailp(); epip()
                    tailp, epip = attn_pass(1, 0)
                    tailp(); epip()
                    tailp, epip = attn_pass(1, 1, last=True)
                    tailp(); epip()
                    out_proj(1, last=True)
    nc.compile()
    return nc


_NC = None


def _get_nc():
    global _NC
    if _NC is None:
        _NC = build_nc()
    return _NC


def make_in_maps(x, Wq, bq, Wk, bk, Wv, bv, Wo, bo):
    B = x.shape[0]
    xs = np.ascontiguousarray(x.reshape(B, C, N).astype(np.float32, copy=False))
    shared = {
        "wqt": np.ascontiguousarray(Wq.T.astype(np.float32, copy=False)),
        "wkt": np.ascontiguousarray(Wk.T.astype(np.float32, copy=False)),
        "wvt": np.ascontiguousarray(Wv.T.astype(np.float32, copy=False)),
        "wot": np.ascontiguousarray(Wo.T.astype(np.float32, copy=False)),
        "bq": np.ascontiguousarray(bq.astype(np.float32, copy=False)),
        "bk": np.ascontiguousarray(bk.astype(np.float32, copy=False)),
        "bv": np.ascontiguousarray(bv.astype(np.float32, copy=False)),
        "bo": np.ascontiguousarray(bo.astype(np.float32, copy=False)),
    }
    return [dict(shared, x=xs[c]) for c in range(B)]


def kernel(x, Wq, bq, Wk, bk, Wv, bv, Wo, bo, **run_kwargs):
    x = np.asarray(x)
    B, _, H, W = x.shape
    in_maps = make_in_maps(
        x, np.asarray(Wq), np.asarray(bq), np.asarray(Wk), np.asarray(bk),
        np.asarray(Wv), np.asarray(bv), np.asarray(Wo), np.asarray(bo),
    )
    res = run_bass_kernel_spmd(_get_nc(), in_maps, core_ids=list(range(B)),
                               **run_kwargs)
    y = np.stack([res.results[c]["y"] for c in range(B)])
    out = y.reshape(B, C, H, W)
    if run_kwargs:
        kernel.last_result = res
    return out

